# revision 71
# baseline (speedup 1.0000x reference)
"""Trainium2 Bass kernel for thresholded multi-head attention.

Computes, for x:[b,n,dim] with b=4, n=2048, dim=512, heads=8, dh=64:
    qkv = x @ Wqkv + bqkv ; split q,k,v per head
    dots = q k^T / sqrt(dh) ; attn = softmax(dots)
    attn = where(attn > 0.01, attn, 0) ; out = attn @ v
    return out @ Wout + bout

Sharding over 8 NeuronCores: core c handles batch b = c//2 and head group
g = c%2 (4 of the 8 heads), producing a partial output projection for its
batch; host sums the two partials per batch and adds bout.

Numerics (zero threshold flips required: min |w/0.01-1| on this data is
1.5e-6 and a single flip contributes ~1e-2 absmax vs a 2.8e-3 budget):
  - q/k projection: 3-limb fp16 (Wh*xh + Wl*xh + Wh*xl), fp32 PSUM;
    x limbs are pre-transposed on the host so no PE transposes are spent.
  - S^T logits: FULL fp32-exact product via 2 K-stacked fp16 matmuls:
    [k_hi;k_lo]^T [q_hi;q_hi] + [k_hi;k_lo]^T [q_lo;q_lo]. Same PE cost
    as a 3-limb scheme but exactly (k_hi+k_lo)*(q_hi+q_lo).
  - exp in fp32 on Scalar (no max-subtraction; unit-variance logits).
  - Z: exact-fp32 pairwise add tree over the E tiles (DVE + GPSIMD +
    accumulate-DMAs), collapsed across the 128 key partitions by
    GPSIMD's partition_all_reduce, which leaves Z broadcast on every
    partition; a Scalar copy stages it into PSUM so the masks read it
    via the DVE's PSUM port. The threshold compare (0.01*Z, in fp32)
    is fused into a one-pass custom DVE select; PV in fp16; the 1/Z
    scale uses the fast DVE reciprocal (values only, not the compare).
Input x limbs ride the Scalar engine's HWDGE DMA queue in parallel
with the weight loads on sync, shrinking the load head. Iterations run
qc-major so each query chunk's out-projection (+ output DMA) interleaves
into the attention loop as soon as its last head completes, removing the
serial phase-E tail and feeding the PE extra work in its gap windows.
Measured vs the CPU fp32 reference: absmax error 7.3e-5 (0.05% of the
output absmax), zero threshold flips; HW exec ~525-531 us vs the 652 us
of the previous Z-limb/gpsimd-residual version.
"""
import os
import sys
import functools

import numpy as np

for _p in ("/opt/trn_rl_repo", "/root/.axon_site", "/root/.axon_site/_ro/trn_rl_repo"):
    if os.path.isdir(_p) and _p not in sys.path:
        sys.path.append(_p)

from contextlib import ExitStack

import concourse.bass as bass
import concourse.bacc as bacc
import concourse.mybir as mybir
import concourse.tile as tile
from concourse import bass_utils

FP32 = mybir.dt.float32
FP16 = mybir.dt.float16
F32R = mybir.dt.float32r
ALU = mybir.AluOpType
AFT = mybir.ActivationFunctionType


def _register_mask_op():
    """One-pass masked keep: out = in0 if in1 < in0 else 0.

    Registered through the documented custom-DVE extension point
    (dve_ops.OPS); used with in0 = E (fp32) and in1 = broadcast threshold.
    """
    from concourse.dve_spec import Spec, Src0, Src1, Zero, select
    from concourse import dve_ops as dops

    name = "MASK_KEEP_GT_ANT"
    for op in dops.OPS:
        if op.name == name:
            return op
    op = dops.DveOp(
        name,
        Spec(
            body=select(Src1 < Src0, Src0, Zero),
            reference=lambda in0, in1, s0, s1, imm2: np.where(
                in1 < in0, in0, 0.0).astype(np.float32),
        ),
        subdim=False,
        uops_sha={"v3": "d86f8416d0d7b042", "v4": "f70e64aee8639ca3"},
    )
    dops.OPS.append(op)
    dops._SUB_OPCODE_FOR_NAME[name] = dops._CUSTOM_DVE_ROW_BASE + len(dops.OPS) - 1
    dops.CUSTOM_DVE_SPECS[name] = op.spec
    return op


def _register_mask_scaled_op():
    """One-pass scaled masked keep: out = in0 if in1*imm2 < in0 else 0.

    Used with in0 = E (fp32), in1 = Z broadcast across partitions and
    imm2 = 0.01: keeps exactly the attention weights above the threshold,
    with the 0.01*Z product evaluated in fp32 inside the DVE."""
    from concourse.dve_spec import Spec, Src0, Src1, Zero, C0, select
    from concourse import dve_ops as dops

    name = "MASK_PPS_GT_ANT"
    for op in dops.OPS:
        if op.name == name:
            return op
    op = dops.DveOp(
        name,
        Spec(
            body=select(Src1 * C0 < Src0, Src0, Zero),
            reference=lambda in0, in1, s0, s1, imm2: np.where(
                np.asarray(in1).reshape(np.asarray(in0).shape) * s0 < in0,
                in0, 0.0).astype(np.float32),
        ),
        subdim=False,
        uops_sha={"v3": "3b3498f864d57883", "v4": "8293d2d8d395f5a1"},
        perf_en={"v3": True, "v4": True},
    )
    dops.OPS.append(op)
    dops._SUB_OPCODE_FOR_NAME[name] = dops._CUSTOM_DVE_ROW_BASE + len(dops.OPS) - 1
    dops.CUSTOM_DVE_SPECS[name] = op.spec
    return op


MASK_OP = _register_mask_op()
MASK2_OP = _register_mask_scaled_op()


def emit_core_kernel(ctx, tc, io, n=2048, dim=512, hc=4, dh=64, qch=512):
    """Emit one core's program. io: dict name -> bass.AP (DRAM)."""
    nc = tc.nc
    inner = hc * dh                 # 256
    NT = n // 128                   # row tiles of n
    KT = n // 128                   # key tiles per (h, qc)
    QC = n // qch                   # query chunks per head
    DC = dim // 128                 # contraction chunks of dim
    MQK = 2 * inner // 128          # m-tiles of stacked [q;k] dims (4)
    MH = inner // 128               # m-tiles of attn-out dims (2)
    NQ = n // qch                   # 512-wide n chunks in phase B
    scale = dh ** -0.5

    # ---------------- persistent constants ----------------
    cpool = ctx.enter_context(tc.tile_pool(name="consts", bufs=1))

    # persistent activations
    apool = ctx.enter_context(tc.tile_pool(name="acts", bufs=1))
    qhh = [apool.tile([128, n], FP16, tag=f"qhh{h}", name=f"qhh{h}") for h in range(hc)]
    qll = [apool.tile([128, n], FP16, tag=f"qll{h}", name=f"qll{h}") for h in range(hc)]
    kstk = [apool.tile([128, n], FP16, tag=f"kstk{h}", name=f"kstk{h}") for h in range(hc)]
    V_sb = [apool.tile([128, inner], FP16, tag=f"V{t}", name=f"V{t}") for t in range(NT)]
    attnB = [apool.tile([128, n], FP16, tag=f"attnB{m}", name=f"attnB{m}") for m in range(MH)]

    # ---------------- phase B: projections (xT limbs DMA'd pre-transposed) --
    # weights/biases live in this pool too: all are dead after phase B
    with tc.tile_pool(name="xT", bufs=1) as xtp:
        wqk_h = []
        wqk_x = []
        wv_h = []
        for c in range(DC):
            t = xtp.tile([128, 2 * inner], FP16, tag=f"wqkh{c}", name=f"wqkh{c}")
            nc.sync.dma_start(t[:], io["wqk_h"][c * 128:(c + 1) * 128, :])
            wqk_h.append(t)
            t = xtp.tile([128, inner], FP16, tag=f"wvh{c}", name=f"wvh{c}")
            nc.sync.dma_start(t[:], io["wv_h"][c * 128:(c + 1) * 128, :])
            wv_h.append(t)
        for c in range(2 * DC):
            t = xtp.tile([128, 2 * inner], FP16, tag=f"wqkx{c}", name=f"wqkx{c}")
            nc.sync.dma_start(t[:], io["wqk_x"][c * 128:(c + 1) * 128, :])
            wqk_x.append(t)
        # per-head biases, duplicated across both 64-partition halves so
        # every engine op reads its bias at the partition of its output
        bq2 = []
        bk2 = []
        for h in range(hc):
            t = xtp.tile([128, 1], FP32, tag=f"bq2{h}", name=f"bq2{h}")
            nc.sync.dma_start(t[:], io["bqk2"][h * 128:(h + 1) * 128, :])
            bq2.append(t)
            t = xtp.tile([128, 1], FP32, tag=f"bk2{h}", name=f"bk2{h}")
            nc.sync.dma_start(t[:], io["bqk2"][(hc + h) * 128:(hc + h + 1) * 128, :])
            bk2.append(t)
        bv_row = xtp.tile([1, inner], FP16, tag="bv", name="bv_row")
        nc.sync.dma_start(bv_row[:], io["bv"][:])
        ones_row16 = xtp.tile([1, 128], FP16, tag="ones_row16", name="ones_row16")
        nc.vector.memset(ones_row16[:], 1.0)
        # chunked so phase-B matmuls can start after the first n-chunk lands
        xTh = [xtp.tile([128, n], FP16, tag=f"xTh{c}", name=f"xTh{c}") for c in range(DC)]
        xTl = [xtp.tile([128, n], FP16, tag=f"xTl{c}", name=f"xTl{c}") for c in range(DC)]
        # xT bulk rides the Scalar engine's HWDGE queue (idle until the
        # first exp) so it streams in parallel with the weight loads on sync
        for nq in range(NQ):
            sl = slice(nq * qch, (nq + 1) * qch)
            for c in range(DC):
                nc.scalar.dma_start(xTh[c][:, sl], io["xt_h"][c * 128:(c + 1) * 128, sl])
                nc.scalar.dma_start(xTl[c][:, sl], io["xt_l"][c * 128:(c + 1) * 128, sl])
        wout = []
        for m in range(MH):
            t = cpool.tile([128, dim], FP16, tag=f"wout{m}", name=f"wout{m}")
            nc.sync.dma_start(t[:], io["wout_b"][m * 128:(m + 1) * 128, :])
            wout.append(t)

        with tc.tile_pool(name="psB", bufs=4, space="PSUM") as psB:
            # qkT = (Wqk^T x^T) -> per-head stacked limb tiles, with bias
            for m in range(MQK):
                msl = slice(m * 128, (m + 1) * 128)
                for nq in range(NQ):
                    sl = slice(nq * qch, (nq + 1) * qch)
                    ps = psB.tile([128, qch], FP32, tag="psB")
                    for c in range(DC):
                        nc.tensor.matmul(ps[:], wqk_h[c][:, msl], xTh[c][:, sl],
                                         start=(c == 0), stop=False)
                    for c2 in range(2 * DC):
                        rhs = xTh[c2][:, sl] if c2 < DC else xTl[c2 - DC][:, sl]
                        nc.tensor.matmul(ps[:], wqk_x[c2][:, msl], rhs,
                                         start=False, stop=(c2 == 2 * DC - 1))
                    is_q = m < MH
                    for hi_half in range(2):      # which head within the m-tile
                        h = 2 * (m % MH) + hi_half
                        pr = slice(64 * hi_half, 64 * hi_half + 64)
                        if is_q:
                            # hi / lo limbs into the low halves; the high
                            # halves are duplicated by SBUF->SBUF DMA below
                            nc.scalar.activation(qhh[h][0:64, sl], ps[pr, :],
                                                 AFT.Identity,
                                                 bias=bq2[h][0:64])
                            nc.vector.scalar_tensor_tensor(
                                qll[h][0:64, sl], ps[pr, :], bq2[h][0:64],
                                qhh[h][0:64, sl], ALU.add, ALU.subtract)
                        else:
                            # kstk = [k_hi; k_lo]: hi into rows 0:64, DMA-dup
                            # into 64:128, then overwrite in place with the
                            # residual (in1 start partition == out's).
                            nc.scalar.activation(kstk[h][0:64, sl], ps[pr, :],
                                                 AFT.Identity,
                                                 bias=bk2[h][0:64])
                            nc.sync.dma_start(kstk[h][64:128, sl],
                                              kstk[h][0:64, sl])
                            nc.vector.scalar_tensor_tensor(
                                kstk[h][64:128, sl], ps[pr, :], bk2[h][64:128],
                                kstk[h][64:128, sl], ALU.add, ALU.subtract)
            # duplicate q limb tiles into their high halves
            for h in range(hc):
                nc.sync.dma_start(qhh[h][64:128, :], qhh[h][0:64, :])
                nc.sync.dma_start(qll[h][64:128, :], qll[h][0:64, :])
            # V natural [n, inner] in fp16, bias via rank-1 ones
            for nt in range(NT):
                tsl = slice(nt * 128, (nt + 1) * 128)
                psv = psB.tile([128, inner], FP32, tag="psV")
                for c in range(DC):
                    nc.tensor.matmul(psv[:], xTh[c][:, tsl], wv_h[c][:],
                                     start=(c == 0), stop=False)
                nc.tensor.matmul(psv[:], ones_row16[:], bv_row[:],
                                 start=False, stop=True)
                nc.scalar.activation(V_sb[nt][:], psv[:], AFT.Copy)

    # ---------------- phase C: attention ----------------
    # Software pipeline, 2-deep: iteration i emits S/exp; i-1 emits the
    # Z-sum tree, the partition all-reduce and masks; i-2 emits PV + 1/Z
    # scale. The PE stream is [S(i) | PV(i-2)] and never waits on
    # DVE/GPSIMD-produced tiles from the current iteration. Z is summed
    # off the PE entirely: an fp32 add tree split across GPSIMD and DVE,
    # collapsed across partitions by GPSIMD's partition_all_reduce, which
    # leaves Z broadcast on all 128 partitions for the threshold compare.
    ST = KT // 2                    # S/E tiles per iteration (2 key-tiles each)
    with tc.tile_pool(name="psS", bufs=2, space="PSUM") as psSp, \
         tc.tile_pool(name="psO", bufs=1, space="PSUM") as psOp, \
         tc.tile_pool(name="psZb", bufs=1, space="PSUM") as psZbp, \
         tc.tile_pool(name="Epool", bufs=2 * ST, space="SBUF") as Ep, \
         tc.tile_pool(name="Ppool", bufs=2 * ST, space="SBUF") as Pp, \
         tc.tile_pool(name="zsum", bufs=1) as zsp, \
         tc.tile_pool(name="zb", bufs=2) as zbp, \
         tc.tile_pool(name="psE", bufs=2, space="PSUM") as psEp, \
         tc.tile_pool(name="ostage", bufs=2) as osp:

        def stage_S(h, qc):
            """S^T matmuls (fp32-exact via K-stacked fp16 limb pairs) + exp."""
            qsl = slice(qc * qch, (qc + 1) * qch)
            E_tiles = []
            for t in range(ST):
                ps = psSp.tile([128, 2 * qch], FP32, tag="S")
                for j in range(2):
                    ksl = slice((2 * t + j) * 128, (2 * t + j + 1) * 128)
                    out = ps[:, j * qch:(j + 1) * qch]
                    nc.tensor.matmul(out, kstk[h][:, ksl], qhh[h][:, qsl],
                                     start=True, stop=False)
                    nc.tensor.matmul(out, kstk[h][:, ksl], qll[h][:, qsl],
                                     start=False, stop=True)
                Et = Ep.tile([128, 2 * qch], FP32, tag="E")
                nc.scalar.activation(Et[:], ps[:], AFT.Exp, scale=scale)
                E_tiles.append(Et)
            return E_tiles

        def stage_Z(state):
            """Z-sum, emitted with its own iteration so it pipelines behind
            exp: the GPSIMD running tile takes the first pair (available
            earliest), DVE the other three; levels 2-3 are accumulate-DMAs
            chained on the GPSIMD tile; partition all-reduce broadcasts Z,
            and a sync-queue DMA stages it into PSUM (so the mask reads it
            via the DVE's PSUM port, off the SBUF read port)."""
            E = state["E"]
            u0 = zsp.tile([128, 2 * qch], FP32, tag="zu0", bufs=2)
            nc.vector.tensor_tensor(u0[:], E[0][:], E[1][:], ALU.add)
            u1 = zsp.tile([128, 2 * qch], FP32, tag="zu1")
            nc.vector.tensor_tensor(u1[:], E[2][:], E[3][:], ALU.add)
            u2 = zsp.tile([128, 2 * qch], FP32, tag="zu2", bufs=2)
            nc.vector.tensor_tensor(u2[:], E[4][:], E[5][:], ALU.add)
            u3 = zsp.tile([128, 2 * qch], FP32, tag="zu3")
            nc.gpsimd.tensor_tensor(u3[:], E[6][:], E[7][:], ALU.add)
            nc.gpsimd.dma_start(u0[:], u1[:], accum_op=ALU.add)
            nc.gpsimd.dma_start(u2[:], u3[:], accum_op=ALU.add)
            nc.gpsimd.dma_start(u0[:], u2[:], accum_op=ALU.add)
            zpre = zsp.tile([128, qch], FP32, tag="zpre", bufs=2)
            nc.vector.tensor_tensor(zpre[:], u0[:, 0:qch], u0[:, qch:2 * qch],
                                    ALU.add)
            Zb = zbp.tile([128, qch], FP32, tag="Zb")
            nc.gpsimd.partition_all_reduce(Zb[:], zpre[:], 128,
                                           bass.bass_isa.ReduceOp.add)
            # stage Z into PSUM so the mask's second operand reads via the
            # PSUM port instead of contending for the DVE's SBUF read port
            Zp = psZbp.tile([128, qch], FP32, tag="Zp")
            nc.scalar.activation(Zp[:], Zb[:], AFT.Copy)
            state["Zb"] = Zp

        def stage_mask(state):
            """1/Z + thresholded keep (0.01*Z folded into the DVE op); one
            DVE op per E tile, Z broadcast over both halves via a
            stride-0 middle dim."""
            Zb = state["Zb"]
            rb = zbp.tile([128, qch], FP32, tag="rb")
            nc.vector.reciprocal_approx_fast(out=rb[:], in_=Zb[:])
            state["rb"] = rb
            zb3 = Zb[:].unsqueeze(1).broadcast_to([128, 2, qch])
            P_tiles = []
            for t in range(ST):
                Pt = Pp.tile([128, 2 * qch], FP16, tag="P")
                nc.vector._custom_dve(
                    MASK2_OP,
                    out=Pt[:].rearrange("p (r q) -> p r q", r=2),
                    in0=state["E"][t][:].rearrange("p (r q) -> p r q", r=2),
                    in1=zb3, s0=0.01)
                P_tiles.append(Pt)
            state["P"] = P_tiles

        def stage_PV(state):
            """PV accumulation + 1/Z scale into attnB."""
            h, qc = state["hq"]
            qsl = slice(qc * qch, (qc + 1) * qch)
            hsl = slice(h * dh, (h + 1) * dh)
            mq, rq = h // 2, 64 * (h % 2)
            psO = psOp.tile([64, qch], FP32, tag="O")
            for t in range(ST):
                for j in range(2):
                    kt = 2 * t + j
                    nc.tensor.matmul(psO[:], V_sb[kt][:, hsl],
                                     state["P"][t][:, j * qch:(j + 1) * qch],
                                     start=(kt == 0), stop=(kt == KT - 1))
            nc.vector.tensor_tensor(attnB[mq][rq:rq + 64, qsl], psO[:],
                                    state["rb"][rq:rq + 64, :], ALU.mult)

        # 2-deep skew: PE stream per iteration is [S(i) | PV(i-2)]; the
        # DVE/GPSIMD Z-tree and masks of iteration i-1 run behind exp and
        # have a full iteration of slack, so the PE never waits on them.
        def stage_E_chunk(qc):
            """out-projection for the 4 row-tiles of a completed qc chunk,
            interleaved into the attention loop (qc-major order means all
            heads of chunk qc have written attnB by the time this runs)."""
            for nt in range(4 * qc, 4 * qc + 4):
                tsl = slice(nt * 128, (nt + 1) * 128)
                ps = psEp.tile([128, dim], FP32, tag="psE")
                for m in range(MH):
                    nc.tensor.matmul(ps[:], attnB[m][:, tsl], wout[m][:],
                                     start=(m == 0), stop=(m == MH - 1))
                ot = osp.tile([128, dim], FP32, tag="ostage")
                # scalar-only eviction: the DVE is the attention pacer
                nc.scalar.activation(ot[:], ps[:], AFT.Copy)
                nc.sync.dma_start(io["out"][tsl, :], ot[:])

        # qc-major: all 4 heads of a query chunk complete consecutively so
        # its out-projection can interleave with later attention iterations
        iters = [(h, qc) for qc in range(QC) for h in range(hc)]
        states = []
        for i, (h, qc) in enumerate(iters):
            states.append({"hq": (h, qc), "E": stage_S(h, qc)})
            if i >= 1:
                stage_Z(states[i - 1])
                stage_mask(states[i - 1])
            if i >= 2:
                stage_PV(states[i - 2])
                states[i - 2] = None
                if (i - 2) % hc == hc - 1:
                    stage_E_chunk((i - 2) // hc)
        stage_PV(states[-2])
        stage_Z(states[-1])
        stage_mask(states[-1])
        stage_PV(states[-1])
        stage_E_chunk(QC - 1)



def build_program(n=2048, dim=512, hc=4, dh=64, qch=512):
    nc = bacc.Bacc(trn_type="TRN2", target_bir_lowering=False, debug=False)
    inner = hc * dh
    io = {}

    def din(name, shape, dt):
        io[name] = nc.dram_tensor(name, shape, dt, kind="ExternalInput").ap()

    din("xt_h", [dim, n], FP16)
    din("xt_l", [dim, n], FP16)
    din("wqk_h", [dim, 2 * inner], FP16)
    din("wqk_x", [2 * dim, 2 * inner], FP16)
    din("wv_h", [dim, inner], FP16)
    din("bqk2", [2 * hc * 128, 1], FP32)
    din("bv", [1, inner], FP16)
    din("wout_b", [inner, dim], FP16)
    io["out"] = nc.dram_tensor("out", [n, dim], FP32, kind="ExternalOutput").ap()

    with tile.TileContext(nc) as tc:
        with ExitStack() as ctx:
            emit_core_kernel(ctx, tc, io, n=n, dim=dim, hc=hc, dh=dh, qch=qch)
    nc.compile()
    return nc


def make_core_inputs(x_b, Wq, Wk, Wv, bq, bk, bv, Wout_g, n=2048, dim=512,
                     hc=4, dh=64):
    """Host-side prep of one core's input dict (numpy, correct dtypes)."""
    f16 = np.float16
    inner = hc * dh
    xh = x_b.astype(f16)
    xl = (x_b - xh.astype(np.float32)).astype(f16)
    wqk = np.concatenate([Wq, Wk], axis=1)              # [dim, 2*inner]
    wqk_hi = wqk.astype(f16)
    wqk_lo = (wqk - wqk_hi.astype(np.float32)).astype(f16)
    wqk_x = np.concatenate([wqk_lo, wqk_hi], axis=0)    # [2*dim, 2*inner]
    # per-head biases duplicated across both 64-row halves: [bq_h;bq_h] x hc
    # then [bk_h;bk_h] x hc
    bq_h = bq.reshape(hc, dh)
    bk_h = bk.reshape(hc, dh)
    bqk2 = np.concatenate([np.concatenate([bq_h[h], bq_h[h]]) for h in range(hc)]
                          + [np.concatenate([bk_h[h], bk_h[h]]) for h in range(hc)])
    return {
        "xt_h": np.ascontiguousarray(xh.T),
        "xt_l": np.ascontiguousarray(xl.T),
        "wqk_h": wqk_hi, "wqk_x": wqk_x,
        "wv_h": Wv.astype(f16),
        "bqk2": bqk2.reshape(2 * hc * 128, 1).astype(np.float32),
        "bv": bv.reshape(1, inner).astype(f16),
        "wout_b": Wout_g.astype(f16),
    }


@functools.lru_cache(maxsize=1)
def _cached_program():
    return build_program()


def kernel(x, Wqkv, bqkv, Wout, bout):
    x = np.asarray(x, dtype=np.float32)
    Wqkv = np.asarray(Wqkv, dtype=np.float32)
    bqkv = np.asarray(bqkv, dtype=np.float32)
    Wout = np.asarray(Wout, dtype=np.float32)
    bout = np.asarray(bout, dtype=np.float32)

    b, n, dim = x.shape
    H, dh = 8, 64
    inner = H * dh
    hc = 4  # heads per core
    Wq, Wk, Wv = Wqkv[:, :inner], Wqkv[:, inner:2 * inner], Wqkv[:, 2 * inner:]
    bq, bk, bv = bqkv[:inner], bqkv[inner:2 * inner], bqkv[2 * inner:]

    in_maps = []
    for c in range(8):
        bb, g = c // 2, c % 2
        hsl = slice(g * hc * dh, (g + 1) * hc * dh)
        in_maps.append(make_core_inputs(
            x[bb], Wq[:, hsl], Wk[:, hsl], Wv[:, hsl],
            bq[hsl], bk[hsl], bv[hsl], Wout[hsl, :],
            n=n, dim=dim, hc=hc, dh=dh))

    nc = _cached_program()
    res = bass_utils.run_bass_kernel_spmd(nc, in_maps, core_ids=list(range(8)))
    global LAST_RESULTS
    LAST_RESULTS = res
    out = np.empty((b, n, dim), dtype=np.float32)
    for bb in range(b):
        out[bb] = res.results[2 * bb]["out"] + res.results[2 * bb + 1]["out"] \
            + bout
    return out


# revision 72
# speedup vs baseline: 1.0016x; 1.0016x over previous
"""Trainium2 Bass kernel for thresholded multi-head attention.

Computes, for x:[b,n,dim] with b=4, n=2048, dim=512, heads=8, dh=64:
    qkv = x @ Wqkv + bqkv ; split q,k,v per head
    dots = q k^T / sqrt(dh) ; attn = softmax(dots)
    attn = where(attn > 0.01, attn, 0) ; out = attn @ v
    return out @ Wout + bout

Sharding over 8 NeuronCores: core c handles batch b = c//2 and head group
g = c%2 (4 of the 8 heads), producing a partial output projection for its
batch; host sums the two partials per batch and adds bout.

Numerics (zero threshold flips required: min |w/0.01-1| on this data is
1.5e-6 and a single flip contributes ~1e-2 absmax vs a 2.8e-3 budget):
  - q/k projection: 3-limb fp16 (Wh*xh + Wl*xh + Wh*xl), fp32 PSUM;
    x limbs are pre-transposed on the host so no PE transposes are spent.
  - S^T logits: FULL fp32-exact product via 2 K-stacked fp16 matmuls:
    [k_hi;k_lo]^T [q_hi;q_hi] + [k_hi;k_lo]^T [q_lo;q_lo]. Same PE cost
    as a 3-limb scheme but exactly (k_hi+k_lo)*(q_hi+q_lo).
  - exp in fp32 on Scalar (no max-subtraction; unit-variance logits).
  - Z: exact-fp32 pairwise add tree over the E tiles (DVE + GPSIMD +
    accumulate-DMAs), collapsed across the 128 key partitions by
    GPSIMD's partition_all_reduce, which leaves Z broadcast on every
    partition; a Scalar copy stages it into PSUM so the masks read it
    via the DVE's PSUM port. The threshold compare (0.01*Z, in fp32)
    is fused into a one-pass custom DVE select; PV in fp16; the 1/Z
    scale uses the fast DVE reciprocal (values only, not the compare).
Input x limbs ride the Scalar engine's HWDGE DMA queue in parallel
with the weight loads on sync, shrinking the load head. Iterations run
qc-major so each query chunk's out-projection (+ output DMA) interleaves
into the attention loop as soon as its last head completes, removing the
serial phase-E tail and feeding the PE extra work in its gap windows.
Measured vs the CPU fp32 reference: absmax error 7.3e-5 (0.05% of the
output absmax), zero threshold flips; HW exec ~525-531 us vs the 652 us
of the previous Z-limb/gpsimd-residual version.
"""
import os
import sys
import functools

import numpy as np

for _p in ("/opt/trn_rl_repo", "/root/.axon_site", "/root/.axon_site/_ro/trn_rl_repo"):
    if os.path.isdir(_p) and _p not in sys.path:
        sys.path.append(_p)

from contextlib import ExitStack

import concourse.bass as bass
import concourse.bacc as bacc
import concourse.mybir as mybir
import concourse.tile as tile
from concourse import bass_utils

FP32 = mybir.dt.float32
FP16 = mybir.dt.float16
F32R = mybir.dt.float32r
ALU = mybir.AluOpType
AFT = mybir.ActivationFunctionType


def _register_mask_op():
    """One-pass masked keep: out = in0 if in1 < in0 else 0.

    Registered through the documented custom-DVE extension point
    (dve_ops.OPS); used with in0 = E (fp32) and in1 = broadcast threshold.
    """
    from concourse.dve_spec import Spec, Src0, Src1, Zero, select
    from concourse import dve_ops as dops

    name = "MASK_KEEP_GT_ANT"
    for op in dops.OPS:
        if op.name == name:
            return op
    op = dops.DveOp(
        name,
        Spec(
            body=select(Src1 < Src0, Src0, Zero),
            reference=lambda in0, in1, s0, s1, imm2: np.where(
                in1 < in0, in0, 0.0).astype(np.float32),
        ),
        subdim=False,
        uops_sha={"v3": "d86f8416d0d7b042", "v4": "f70e64aee8639ca3"},
    )
    dops.OPS.append(op)
    dops._SUB_OPCODE_FOR_NAME[name] = dops._CUSTOM_DVE_ROW_BASE + len(dops.OPS) - 1
    dops.CUSTOM_DVE_SPECS[name] = op.spec
    return op


def _register_mask_scaled_op():
    """One-pass scaled masked keep: out = in0 if in1*imm2 < in0 else 0.

    Used with in0 = E (fp32), in1 = Z broadcast across partitions and
    imm2 = 0.01: keeps exactly the attention weights above the threshold,
    with the 0.01*Z product evaluated in fp32 inside the DVE."""
    from concourse.dve_spec import Spec, Src0, Src1, Zero, C0, select
    from concourse import dve_ops as dops

    name = "MASK_PPS_GT_ANT"
    for op in dops.OPS:
        if op.name == name:
            return op
    op = dops.DveOp(
        name,
        Spec(
            body=select(Src1 * C0 < Src0, Src0, Zero),
            reference=lambda in0, in1, s0, s1, imm2: np.where(
                np.asarray(in1).reshape(np.asarray(in0).shape) * s0 < in0,
                in0, 0.0).astype(np.float32),
        ),
        subdim=False,
        uops_sha={"v3": "3b3498f864d57883", "v4": "8293d2d8d395f5a1"},
        perf_en={"v3": True, "v4": True},
    )
    dops.OPS.append(op)
    dops._SUB_OPCODE_FOR_NAME[name] = dops._CUSTOM_DVE_ROW_BASE + len(dops.OPS) - 1
    dops.CUSTOM_DVE_SPECS[name] = op.spec
    return op


MASK_OP = _register_mask_op()
MASK2_OP = _register_mask_scaled_op()


def emit_core_kernel(ctx, tc, io, n=2048, dim=512, hc=4, dh=64, qch=512):
    """Emit one core's program. io: dict name -> bass.AP (DRAM)."""
    nc = tc.nc
    inner = hc * dh                 # 256
    NT = n // 128                   # row tiles of n
    KT = n // 128                   # key tiles per (h, qc)
    QC = n // qch                   # query chunks per head
    DC = dim // 128                 # contraction chunks of dim
    MQK = 2 * inner // 128          # m-tiles of stacked [q;k] dims (4)
    MH = inner // 128               # m-tiles of attn-out dims (2)
    NQ = n // qch                   # 512-wide n chunks in phase B
    scale = dh ** -0.5

    # ---------------- persistent constants ----------------
    cpool = ctx.enter_context(tc.tile_pool(name="consts", bufs=1))

    # persistent activations
    apool = ctx.enter_context(tc.tile_pool(name="acts", bufs=1))
    qhh = [apool.tile([128, n], FP16, tag=f"qhh{h}", name=f"qhh{h}") for h in range(hc)]
    qll = [apool.tile([128, n], FP16, tag=f"qll{h}", name=f"qll{h}") for h in range(hc)]
    kstk = [apool.tile([128, n], FP16, tag=f"kstk{h}", name=f"kstk{h}") for h in range(hc)]
    V_sb = [apool.tile([128, inner], FP16, tag=f"V{t}", name=f"V{t}") for t in range(NT)]
    attnB = [apool.tile([128, n], FP16, tag=f"attnB{m}", name=f"attnB{m}") for m in range(MH)]

    # ---------------- phase B: projections (xT limbs DMA'd pre-transposed) --
    # weights/biases live in this pool too: all are dead after phase B
    with tc.tile_pool(name="xT", bufs=1) as xtp:
        wqk_h = []
        wqk_x = []
        wv_h = []
        for c in range(DC):
            t = xtp.tile([128, 2 * inner], FP16, tag=f"wqkh{c}", name=f"wqkh{c}")
            nc.sync.dma_start(t[:], io["wqk_h"][c * 128:(c + 1) * 128, :])
            wqk_h.append(t)
            t = xtp.tile([128, inner], FP16, tag=f"wvh{c}", name=f"wvh{c}")
            nc.sync.dma_start(t[:], io["wv_h"][c * 128:(c + 1) * 128, :])
            wv_h.append(t)
        for c in range(2 * DC):
            t = xtp.tile([128, 2 * inner], FP16, tag=f"wqkx{c}", name=f"wqkx{c}")
            nc.sync.dma_start(t[:], io["wqk_x"][c * 128:(c + 1) * 128, :])
            wqk_x.append(t)
        # per-head biases, duplicated across both 64-partition halves so
        # every engine op reads its bias at the partition of its output
        bq2 = []
        bk2 = []
        for h in range(hc):
            t = xtp.tile([128, 1], FP32, tag=f"bq2{h}", name=f"bq2{h}")
            nc.sync.dma_start(t[:], io["bqk2"][h * 128:(h + 1) * 128, :])
            bq2.append(t)
            t = xtp.tile([128, 1], FP32, tag=f"bk2{h}", name=f"bk2{h}")
            nc.sync.dma_start(t[:], io["bqk2"][(hc + h) * 128:(hc + h + 1) * 128, :])
            bk2.append(t)
        bv_row = xtp.tile([1, inner], FP16, tag="bv", name="bv_row")
        nc.sync.dma_start(bv_row[:], io["bv"][:])
        ones_row16 = xtp.tile([1, 128], FP16, tag="ones_row16", name="ones_row16")
        nc.vector.memset(ones_row16[:], 1.0)
        # chunked so phase-B matmuls can start after the first n-chunk lands
        xTh = [xtp.tile([128, n], FP16, tag=f"xTh{c}", name=f"xTh{c}") for c in range(DC)]
        xTl = [xtp.tile([128, n], FP16, tag=f"xTl{c}", name=f"xTl{c}") for c in range(DC)]
        # xT bulk rides the Scalar engine's HWDGE queue (idle until the
        # first exp) so it streams in parallel with the weight loads on sync
        for nq in range(NQ):
            sl = slice(nq * qch, (nq + 1) * qch)
            for c in range(DC):
                nc.scalar.dma_start(xTh[c][:, sl], io["xt_h"][c * 128:(c + 1) * 128, sl])
                nc.scalar.dma_start(xTl[c][:, sl], io["xt_l"][c * 128:(c + 1) * 128, sl])
        wout = []
        for m in range(MH):
            t = cpool.tile([128, dim], FP16, tag=f"wout{m}", name=f"wout{m}")
            nc.sync.dma_start(t[:], io["wout_b"][m * 128:(m + 1) * 128, :])
            wout.append(t)

        with tc.tile_pool(name="psB", bufs=4, space="PSUM") as psB:
            # qkT = (Wqk^T x^T) -> per-head stacked limb tiles, with bias
            for m in range(MQK):
                msl = slice(m * 128, (m + 1) * 128)
                for nq in range(NQ):
                    sl = slice(nq * qch, (nq + 1) * qch)
                    ps = psB.tile([128, qch], FP32, tag="psB")
                    for c in range(DC):
                        nc.tensor.matmul(ps[:], wqk_h[c][:, msl], xTh[c][:, sl],
                                         start=(c == 0), stop=False)
                    for c2 in range(2 * DC):
                        rhs = xTh[c2][:, sl] if c2 < DC else xTl[c2 - DC][:, sl]
                        nc.tensor.matmul(ps[:], wqk_x[c2][:, msl], rhs,
                                         start=False, stop=(c2 == 2 * DC - 1))
                    is_q = m < MH
                    for hi_half in range(2):      # which head within the m-tile
                        h = 2 * (m % MH) + hi_half
                        pr = slice(64 * hi_half, 64 * hi_half + 64)
                        if is_q:
                            # hi / lo limbs into the low halves; the high
                            # halves are duplicated by SBUF->SBUF DMA below
                            nc.scalar.activation(qhh[h][0:64, sl], ps[pr, :],
                                                 AFT.Identity,
                                                 bias=bq2[h][0:64])
                            nc.vector.scalar_tensor_tensor(
                                qll[h][0:64, sl], ps[pr, :], bq2[h][0:64],
                                qhh[h][0:64, sl], ALU.add, ALU.subtract)
                        else:
                            # kstk = [k_hi; k_lo]: hi into rows 0:64, DMA-dup
                            # into 64:128, then overwrite in place with the
                            # residual (in1 start partition == out's).
                            nc.scalar.activation(kstk[h][0:64, sl], ps[pr, :],
                                                 AFT.Identity,
                                                 bias=bk2[h][0:64])
                            nc.sync.dma_start(kstk[h][64:128, sl],
                                              kstk[h][0:64, sl])
                            nc.vector.scalar_tensor_tensor(
                                kstk[h][64:128, sl], ps[pr, :], bk2[h][64:128],
                                kstk[h][64:128, sl], ALU.add, ALU.subtract)
            # duplicate q limb tiles into their high halves
            for h in range(hc):
                nc.sync.dma_start(qhh[h][64:128, :], qhh[h][0:64, :])
                nc.sync.dma_start(qll[h][64:128, :], qll[h][0:64, :])
            # V natural [n, inner] in fp16, bias via rank-1 ones
            for nt in range(NT):
                tsl = slice(nt * 128, (nt + 1) * 128)
                psv = psB.tile([128, inner], FP32, tag="psV")
                for c in range(DC):
                    nc.tensor.matmul(psv[:], xTh[c][:, tsl], wv_h[c][:],
                                     start=(c == 0), stop=False)
                nc.tensor.matmul(psv[:], ones_row16[:], bv_row[:],
                                 start=False, stop=True)
                nc.scalar.activation(V_sb[nt][:], psv[:], AFT.Copy)

    # ---------------- phase C: attention ----------------
    # Software pipeline, 2-deep: iteration i emits S/exp; i-1 emits the
    # Z-sum tree, the partition all-reduce and masks; i-2 emits PV + 1/Z
    # scale. The PE stream is [S(i) | PV(i-2)] and never waits on
    # DVE/GPSIMD-produced tiles from the current iteration. Z is summed
    # off the PE entirely: an fp32 add tree split across GPSIMD and DVE,
    # collapsed across partitions by GPSIMD's partition_all_reduce, which
    # leaves Z broadcast on all 128 partitions for the threshold compare.
    ST = KT // 2                    # S/E tiles per iteration (2 key-tiles each)
    with tc.tile_pool(name="psS", bufs=2, space="PSUM") as psSp, \
         tc.tile_pool(name="psO", bufs=1, space="PSUM") as psOp, \
         tc.tile_pool(name="psZb", bufs=1, space="PSUM") as psZbp, \
         tc.tile_pool(name="Epool", bufs=2 * ST, space="SBUF") as Ep, \
         tc.tile_pool(name="Ppool", bufs=2 * ST, space="SBUF") as Pp, \
         tc.tile_pool(name="zsum", bufs=1) as zsp, \
         tc.tile_pool(name="zb", bufs=2) as zbp, \
         tc.tile_pool(name="psE", bufs=2, space="PSUM") as psEp, \
         tc.tile_pool(name="ostage", bufs=2) as osp:

        def stage_S(h, qc):
            """S^T matmuls (fp32-exact via K-stacked fp16 limb pairs) + exp."""
            qsl = slice(qc * qch, (qc + 1) * qch)
            E_tiles = []
            for t in range(ST):
                ps = psSp.tile([128, 2 * qch], FP32, tag="S")
                for j in range(2):
                    ksl = slice((2 * t + j) * 128, (2 * t + j + 1) * 128)
                    out = ps[:, j * qch:(j + 1) * qch]
                    nc.tensor.matmul(out, kstk[h][:, ksl], qhh[h][:, qsl],
                                     start=True, stop=False)
                    nc.tensor.matmul(out, kstk[h][:, ksl], qll[h][:, qsl],
                                     start=False, stop=True)
                Et = Ep.tile([128, 2 * qch], FP32, tag="E")
                nc.scalar.activation(Et[:], ps[:], AFT.Exp, scale=scale)
                E_tiles.append(Et)
            return E_tiles

        def stage_Z(state):
            """Z-sum, emitted with its own iteration so it pipelines behind
            exp: the GPSIMD running tile takes the first pair (available
            earliest), DVE the other three; levels 2-3 are accumulate-DMAs
            chained on the GPSIMD tile; partition all-reduce broadcasts Z,
            and a sync-queue DMA stages it into PSUM (so the mask reads it
            via the DVE's PSUM port, off the SBUF read port)."""
            E = state["E"]
            u0 = zsp.tile([128, 2 * qch], FP32, tag="zu0", bufs=2)
            nc.vector.tensor_tensor(u0[:], E[0][:], E[1][:], ALU.add)
            u1 = zsp.tile([128, 2 * qch], FP32, tag="zu1")
            nc.vector.tensor_tensor(u1[:], E[2][:], E[3][:], ALU.add)
            u2 = zsp.tile([128, 2 * qch], FP32, tag="zu2", bufs=2)
            nc.vector.tensor_tensor(u2[:], E[4][:], E[5][:], ALU.add)
            u3 = zsp.tile([128, 2 * qch], FP32, tag="zu3")
            nc.gpsimd.tensor_tensor(u3[:], E[6][:], E[7][:], ALU.add)
            nc.gpsimd.dma_start(u0[:], u1[:], accum_op=ALU.add)
            nc.gpsimd.dma_start(u2[:], u3[:], accum_op=ALU.add)
            nc.gpsimd.dma_start(u0[:], u2[:], accum_op=ALU.add)
            zpre = zsp.tile([128, qch], FP32, tag="zpre", bufs=2)
            nc.vector.tensor_tensor(zpre[:], u0[:, 0:qch], u0[:, qch:2 * qch],
                                    ALU.add)
            Zb = zbp.tile([128, qch], FP32, tag="Zb")
            nc.gpsimd.partition_all_reduce(Zb[:], zpre[:], 128,
                                           bass.bass_isa.ReduceOp.add)
            # stage Z into PSUM so the mask's second operand reads via the
            # PSUM port instead of contending for the DVE's SBUF read port
            Zp = psZbp.tile([128, qch], FP32, tag="Zp")
            nc.scalar.activation(Zp[:], Zb[:], AFT.Copy)
            state["Zb"] = Zp

        def stage_mask(state):
            """1/Z + thresholded keep (0.01*Z folded into the DVE op); one
            DVE op per E tile, Z broadcast over both halves via a
            stride-0 middle dim."""
            Zb = state["Zb"]
            rb = zbp.tile([128, qch], FP32, tag="rb")
            nc.vector.reciprocal_approx_fast(out=rb[:], in_=Zb[:])
            state["rb"] = rb
            zb3 = Zb[:].unsqueeze(1).broadcast_to([128, 2, qch])
            P_tiles = []
            for t in range(ST):
                Pt = Pp.tile([128, 2 * qch], FP16, tag="P")
                nc.vector._custom_dve(
                    MASK2_OP,
                    out=Pt[:].rearrange("p (r q) -> p r q", r=2),
                    in0=state["E"][t][:].rearrange("p (r q) -> p r q", r=2),
                    in1=zb3, s0=0.01)
                P_tiles.append(Pt)
            state["P"] = P_tiles

        def stage_PV(state):
            """PV accumulation + 1/Z scale into attnB."""
            h, qc = state["hq"]
            qsl = slice(qc * qch, (qc + 1) * qch)
            hsl = slice(h * dh, (h + 1) * dh)
            mq, rq = h // 2, 64 * (h % 2)
            psO = psOp.tile([64, qch], FP32, tag="O")
            for t in range(ST):
                for j in range(2):
                    kt = 2 * t + j
                    nc.tensor.matmul(psO[:], V_sb[kt][:, hsl],
                                     state["P"][t][:, j * qch:(j + 1) * qch],
                                     start=(kt == 0), stop=(kt == KT - 1))
            nc.vector.tensor_tensor(attnB[mq][rq:rq + 64, qsl], psO[:],
                                    state["rb"][rq:rq + 64, :], ALU.mult)

        # 2-deep skew: PE stream per iteration is [S(i) | PV(i-2)]; the
        # DVE/GPSIMD Z-tree and masks of iteration i-1 run behind exp and
        # have a full iteration of slack, so the PE never waits on them.
        def stage_E_chunk(qc):
            """out-projection for the 4 row-tiles of a completed qc chunk,
            interleaved into the attention loop (qc-major order means all
            heads of chunk qc have written attnB by the time this runs)."""
            for nt in range(4 * qc, 4 * qc + 4):
                tsl = slice(nt * 128, (nt + 1) * 128)
                ps = psEp.tile([128, dim], FP32, tag="psE")
                for m in range(MH):
                    nc.tensor.matmul(ps[:], attnB[m][:, tsl], wout[m][:],
                                     start=(m == 0), stop=(m == MH - 1))
                ot = osp.tile([128, dim], FP32, tag="ostage")
                # scalar-only eviction: the DVE is the attention pacer
                nc.scalar.activation(ot[:], ps[:], AFT.Copy)
                nc.sync.dma_start(io["out"][tsl, :], ot[:])

        # qc-major: all 4 heads of a query chunk complete consecutively so
        # its out-projection can interleave with later attention iterations
        iters = [(h, qc) for qc in range(QC) for h in range(hc)]
        states = []
        for i, (h, qc) in enumerate(iters):
            states.append({"hq": (h, qc), "E": stage_S(h, qc)})
            if i >= 1:
                stage_Z(states[i - 1])
                stage_mask(states[i - 1])
            if i >= 2:
                stage_PV(states[i - 2])
                states[i - 2] = None
                if (i - 2) % hc == hc - 1:
                    stage_E_chunk((i - 2) // hc)
        stage_Z(states[-1])
        stage_mask(states[-1])
        stage_PV(states[-2])
        stage_PV(states[-1])
        stage_E_chunk(QC - 1)



def build_program(n=2048, dim=512, hc=4, dh=64, qch=512):
    nc = bacc.Bacc(trn_type="TRN2", target_bir_lowering=False, debug=False)
    inner = hc * dh
    io = {}

    def din(name, shape, dt):
        io[name] = nc.dram_tensor(name, shape, dt, kind="ExternalInput").ap()

    din("xt_h", [dim, n], FP16)
    din("xt_l", [dim, n], FP16)
    din("wqk_h", [dim, 2 * inner], FP16)
    din("wqk_x", [2 * dim, 2 * inner], FP16)
    din("wv_h", [dim, inner], FP16)
    din("bqk2", [2 * hc * 128, 1], FP32)
    din("bv", [1, inner], FP16)
    din("wout_b", [inner, dim], FP16)
    io["out"] = nc.dram_tensor("out", [n, dim], FP32, kind="ExternalOutput").ap()

    with tile.TileContext(nc) as tc:
        with ExitStack() as ctx:
            emit_core_kernel(ctx, tc, io, n=n, dim=dim, hc=hc, dh=dh, qch=qch)
    nc.compile()
    return nc


def make_core_inputs(x_b, Wq, Wk, Wv, bq, bk, bv, Wout_g, n=2048, dim=512,
                     hc=4, dh=64):
    """Host-side prep of one core's input dict (numpy, correct dtypes)."""
    f16 = np.float16
    inner = hc * dh
    xh = x_b.astype(f16)
    xl = (x_b - xh.astype(np.float32)).astype(f16)
    wqk = np.concatenate([Wq, Wk], axis=1)              # [dim, 2*inner]
    wqk_hi = wqk.astype(f16)
    wqk_lo = (wqk - wqk_hi.astype(np.float32)).astype(f16)
    wqk_x = np.concatenate([wqk_lo, wqk_hi], axis=0)    # [2*dim, 2*inner]
    # per-head biases duplicated across both 64-row halves: [bq_h;bq_h] x hc
    # then [bk_h;bk_h] x hc
    bq_h = bq.reshape(hc, dh)
    bk_h = bk.reshape(hc, dh)
    bqk2 = np.concatenate([np.concatenate([bq_h[h], bq_h[h]]) for h in range(hc)]
                          + [np.concatenate([bk_h[h], bk_h[h]]) for h in range(hc)])
    return {
        "xt_h": np.ascontiguousarray(xh.T),
        "xt_l": np.ascontiguousarray(xl.T),
        "wqk_h": wqk_hi, "wqk_x": wqk_x,
        "wv_h": Wv.astype(f16),
        "bqk2": bqk2.reshape(2 * hc * 128, 1).astype(np.float32),
        "bv": bv.reshape(1, inner).astype(f16),
        "wout_b": Wout_g.astype(f16),
    }


@functools.lru_cache(maxsize=1)
def _cached_program():
    return build_program()


def kernel(x, Wqkv, bqkv, Wout, bout):
    x = np.asarray(x, dtype=np.float32)
    Wqkv = np.asarray(Wqkv, dtype=np.float32)
    bqkv = np.asarray(bqkv, dtype=np.float32)
    Wout = np.asarray(Wout, dtype=np.float32)
    bout = np.asarray(bout, dtype=np.float32)

    b, n, dim = x.shape
    H, dh = 8, 64
    inner = H * dh
    hc = 4  # heads per core
    Wq, Wk, Wv = Wqkv[:, :inner], Wqkv[:, inner:2 * inner], Wqkv[:, 2 * inner:]
    bq, bk, bv = bqkv[:inner], bqkv[inner:2 * inner], bqkv[2 * inner:]

    in_maps = []
    for c in range(8):
        bb, g = c // 2, c % 2
        hsl = slice(g * hc * dh, (g + 1) * hc * dh)
        in_maps.append(make_core_inputs(
            x[bb], Wq[:, hsl], Wk[:, hsl], Wv[:, hsl],
            bq[hsl], bk[hsl], bv[hsl], Wout[hsl, :],
            n=n, dim=dim, hc=hc, dh=dh))

    nc = _cached_program()
    res = bass_utils.run_bass_kernel_spmd(nc, in_maps, core_ids=list(range(8)))
    global LAST_RESULTS
    LAST_RESULTS = res
    out = np.empty((b, n, dim), dtype=np.float32)
    for bb in range(b):
        out[bb] = res.results[2 * bb]["out"] + res.results[2 * bb + 1]["out"] \
            + bout
    return out
